# revision 9
# baseline (speedup 1.0000x reference)
"""Trainium2 Bass kernel for per-edge GNN critic MLP.

q[e] = W2^T relu(W1^T concat(node[src], node[dst], attr, gctx[batch[src]]) + b1) + b2

Strategy (8 cores, edge-parallel):
- Edges sharded 8 ways; node table + weights replicated per core.
- Node table lives in SBUF as bf16 tokens of 128 values: [emb(64) | onehot64(batch)].
  Token id t(n) = (n % 391)*128 + n//391 -> partition n//391, rank n%391.
- Per-edge node rows fetched with SBUF-source transposed dma_gather (free
  transpose into matmul layout). Indices are int16, so edges are bucketed
  host-side by (token_src < 32768, token_dst < 32768) into 4 statically-sized
  groups; gathers address table slice A (ranks 0-255) or B (ranks 256-390).
- ctx contribution folded in as C = gctx @ W1d via the onehot table columns.
- Layer 1: two K-stacked bf16 matmuls per 512-edge tile; relu+b1 on ScalarE.
- Layer 2: W2-stationary bf16 matmul -> psum row; +b2 on copy-out.
"""

import numpy as np

NCORES = 8
V = 50000
VP = 50048          # padded nodes: 128 * 391
KR = 391            # node rows per partition
EMB = 64
EDGE_IN = 16
HIDDEN = 256
NGRAPH = 64
E = 800000
E_CORE = E // NCORES
SPLIT = 32768       # table A tokens (ranks 0-255)
TILE = 512
# group tile capacities (AA, AB, BA, BB); pA = 256/391
CAPS = (86, 46, 46, 25)
TILES = sum(CAPS)   # 203
SLOTS = TILES * TILE

_CACHE = {}


def _build_nc(reps=1):
    import contextlib
    import concourse.bacc as bacc
    import concourse.bass as bass
    import concourse.tile as tile
    from concourse import mybir
    from concourse.masks import make_identity

    f32 = mybir.dt.float32
    bf16 = mybir.dt.bfloat16
    i32 = mybir.dt.int32
    i16 = mybir.dt.int16

    nc = bacc.Bacc("TRN2", debug=False, num_devices=NCORES, num_swdge_queues=4)

    node_d = nc.dram_tensor("node", [128, KR * EMB], f32, kind="ExternalInput")
    batch_d = nc.dram_tensor("batchb", [128, KR], i32, kind="ExternalInput")
    gctx_d = nc.dram_tensor("gctx", [NGRAPH, 2 * EMB], f32, kind="ExternalInput")
    w1_d = nc.dram_tensor("w1", [4 * EMB + EDGE_IN, HIDDEN], f32, kind="ExternalInput")
    b1_d = nc.dram_tensor("b1", [HIDDEN], f32, kind="ExternalInput")
    w2_d = nc.dram_tensor("w2", [HIDDEN], f32, kind="ExternalInput")
    b2_d = nc.dram_tensor("b2", [1], f32, kind="ExternalInput")
    isrc_d = nc.dram_tensor("isrc", [128, TILES * (TILE // 16)], i16, kind="ExternalInput")
    idst_d = nc.dram_tensor("idst", [128, TILES * (TILE // 16)], i16, kind="ExternalInput")
    attr_d = nc.dram_tensor("attr", [EDGE_IN, SLOTS], f32, kind="ExternalInput")
    q_d = nc.dram_tensor("q", [1, SLOTS], f32, kind="ExternalOutput")

    with tile.TileContext(nc) as tc:
        with (
            tc.tile_pool(name="const", bufs=1) as cp,
            tc.tile_pool(name="work", bufs=3) as wp,
            tc.tile_pool(name="qrow", bufs=2) as qp,
            tc.tile_pool(name="psh", bufs=2, space="PSUM") as php,
            tc.tile_pool(name="psq", bufs=2, space="PSUM") as pqp,
        ):
            # ---------------- resident tiles ----------------
            table = cp.tile([128, KR * 128], bf16)          # 97.8 KB/part
            isrc_t = cp.tile([128, TILES * (TILE // 16)], i16)
            idst_t = cp.tile([128, TILES * (TILE // 16)], i16)
            lhs_so = cp.tile([128, HIDDEN], bf16)           # [W1a ; C]
            lhs_da = cp.tile([128, HIDDEN], bf16)           # [W1b ; W1c] (80 rows)
            w2_t = cp.tile([128, 2], bf16)
            b1_t = cp.tile([128, 2], f32)
            b2_t = cp.tile([128, 1], f32)

            nc.sync.dma_start(out=isrc_t[:], in_=isrc_d.ap()[:])
            nc.sync.dma_start(out=idst_t[:], in_=idst_d.ap()[:])

            # ---------------- table build ----------------
            with tc.tile_pool(name="prep", bufs=1) as pp, \
                 tc.tile_pool(name="prepps", bufs=1, space="PSUM") as ppp:
                # node embeddings -> table cols 0-63 of each rank block
                nchunk = 8
                per = KR // nchunk + 1  # 49*7+48
                r0 = 0
                for c in range(nchunk):
                    nr = min(per, KR - r0)
                    stage = pp.tile([128, per * EMB], f32, tag="stage")
                    nc.sync.dma_start(
                        out=stage[:, : nr * EMB],
                        in_=node_d.ap()[:, r0 * EMB:(r0 + nr) * EMB],
                    )
                    dst_view = table[:, r0 * 128:(r0 + nr) * 128].rearrange(
                        "p (r e) -> p r e", e=128
                    )[:, :, 0:EMB]
                    src_view = stage[:, : nr * EMB].rearrange("p (r e) -> p r e", e=EMB)
                    nc.vector.tensor_copy(out=dst_view, in_=src_view)
                    r0 += nr

                # batch -> onehot into table cols 64-127
                batch_i = pp.tile([128, KR], i32, tag="bi")
                batch_b = pp.tile([128, KR], bf16, tag="bb")
                iota_i = pp.tile([128, NGRAPH], i32, tag="ii")
                iota_b = pp.tile([128, NGRAPH], bf16, tag="ib")
                nc.sync.dma_start(out=batch_i[:], in_=batch_d.ap()[:])
                nc.vector.tensor_copy(out=batch_b[:], in_=batch_i[:])
                nc.gpsimd.iota(iota_i[:], pattern=[[1, NGRAPH]], base=0, channel_multiplier=0)
                nc.vector.tensor_copy(out=iota_b[:], in_=iota_i[:])
                oh_view = table[:].rearrange("p (r e) -> p r e", e=128)[:, :, EMB:128]
                nc.vector.tensor_tensor(
                    out=oh_view,
                    in0=batch_b[:][:, :, None].to_broadcast([128, KR, NGRAPH]),
                    in1=iota_b[:][:, None, :].to_broadcast([128, KR, NGRAPH]),
                    op=mybir.AluOpType.is_equal,
                )

                # weights
                wstage = pp.tile([128, HIDDEN], f32, tag="ws")
                nc.sync.dma_start(out=wstage[0:EMB, :], in_=w1_d.ap()[0:EMB, :])
                nc.vector.tensor_copy(out=lhs_so[0:EMB, :], in_=wstage[0:EMB, :])

                wstage2 = pp.tile([128, HIDDEN], f32, tag="ws2")
                nc.sync.dma_start(out=wstage2[0:EMB, :], in_=w1_d.ap()[EMB:2 * EMB, :])
                nc.sync.dma_start(
                    out=wstage2[EMB:EMB + EDGE_IN, :],
                    in_=w1_d.ap()[2 * EMB:2 * EMB + EDGE_IN, :],
                )
                nc.vector.tensor_copy(
                    out=lhs_da[0:EMB + EDGE_IN, :], in_=wstage2[0:EMB + EDGE_IN, :]
                )

                # C = gctx @ W1d  (-> lhs_so rows 64-127)
                gstage = pp.tile([128, 2 * EMB], f32, tag="gs")
                ident = pp.tile([128, NGRAPH], f32, tag="id")
                nc.sync.dma_start(out=gstage[0:NGRAPH, :], in_=gctx_d.ap()[:])
                make_identity(nc, ident[0:NGRAPH, 0:NGRAPH])
                ps_gt = ppp.tile([128, NGRAPH], f32, tag="pgt")
                nc.tensor.transpose(
                    out=ps_gt[:, :], in_=gstage[0:NGRAPH, :], identity=ident[0:NGRAPH, 0:NGRAPH]
                )
                gt_bf = pp.tile([128, NGRAPH], bf16, tag="gt")
                nc.vector.tensor_copy(out=gt_bf[:], in_=ps_gt[:])

                w1d_s = pp.tile([128, HIDDEN], f32, tag="w1ds")
                w1d_b = pp.tile([128, HIDDEN], bf16, tag="w1db")
                nc.sync.dma_start(out=w1d_s[:], in_=w1_d.ap()[2 * EMB + EDGE_IN:, :])
                nc.vector.tensor_copy(out=w1d_b[:], in_=w1d_s[:])
                ps_c = ppp.tile([128, HIDDEN], f32, tag="pc")
                nc.tensor.matmul(
                    out=ps_c[NGRAPH:128, :], lhsT=gt_bf[:], rhs=w1d_b[:],
                    start=True, stop=True, tile_position=(0, 64),
                )
                nc.vector.tensor_copy(out=lhs_so[NGRAPH:128, :], in_=ps_c[NGRAPH:128, :])

                # w2 / b1 / b2
                w2s = pp.tile([128, 2], f32, tag="w2s")
                nc.sync.dma_start(
                    out=w2s[:], in_=w2_d.ap().rearrange("(m p) -> p m", p=128)
                )
                nc.vector.tensor_copy(out=w2_t[:], in_=w2s[:])
                nc.sync.dma_start(
                    out=b1_t[:], in_=b1_d.ap().rearrange("(m p) -> p m", p=128)
                )
                nc.sync.dma_start(out=b2_t[0:1, :], in_=b2_d.ap()[:, None])
                nc.gpsimd.partition_broadcast(b2_t[:], b2_t[0:1, :])

            # ---------------- main loop ----------------
            relu = mybir.ActivationFunctionType.Relu
            IDXC = TILE // 16  # idx cols per tile
            QCHUNK = 4         # tiles per q row buffer
            q_sb = None
            groups = []
            t0 = 0
            for gi, cap in enumerate(CAPS):
                groups.append((t0, t0 + cap, gi // 2, gi % 2))  # [t0,t1), srcB?, dstB?
                t0 += cap

            loop_cm = tc.For_i(0, reps, 1) if reps > 1 else contextlib.nullcontext()
            with loop_cm:
              for (ts, te, src_b, dst_b) in groups:
                for t in range(ts, te):
                    src_t = wp.tile([128, TILE], bf16, tag="src")
                    dst_t = wp.tile([128, TILE], bf16, tag="dst")
                    for (out_t, idx_t, is_b, qn) in (
                        (src_t, isrc_t, src_b, (2 * t) % 4),
                        (dst_t, idst_t, dst_b, (2 * t + 1) % 4),
                    ):
                        if is_b:
                            in_ap = table[:, SPLIT * 1:KR * 128]
                        else:
                            in_ap = table[:, 0:SPLIT * 1]
                        nc.gpsimd.dma_gather(
                            out_ap=out_t[:].rearrange("p (o n) -> p o n", o=1),
                            in_ap=in_ap,
                            idxs_ap=idx_t[:, t * IDXC:(t + 1) * IDXC],
                            num_idxs=TILE, num_idxs_reg=TILE, elem_size=128,
                            transpose=True, sbuf_tokens_per_rank=128,
                            sbuf_free_dim_per_rank=256,
                            queue_num=qn,
                        )

                    # attr -> dst_t rows 64-79 (bf16)
                    attr_s = wp.tile([128, TILE], f32, tag="attr")
                    nc.sync.dma_start(
                        out=attr_s[EMB:EMB + EDGE_IN, :],
                        in_=attr_d.ap()[:, t * TILE:(t + 1) * TILE],
                    )
                    nc.vector.tensor_copy(
                        out=dst_t[EMB:EMB + EDGE_IN, :],
                        in_=attr_s[EMB:EMB + EDGE_IN, :],
                    )

                    # layer 1
                    h_tiles = []
                    for m in range(2):
                        ph = php.tile([128, TILE], f32, tag=f"h{m}")
                        nc.tensor.matmul(
                            out=ph[:], lhsT=lhs_so[:, m * 128:(m + 1) * 128],
                            rhs=src_t[:], start=True, stop=False,
                        )
                        nc.tensor.matmul(
                            out=ph[:],
                            lhsT=lhs_da[0:EMB + EDGE_IN, m * 128:(m + 1) * 128],
                            rhs=dst_t[0:EMB + EDGE_IN, :], start=False, stop=True,
                        )
                        hb = wp.tile([128, TILE], bf16, tag=f"hb{m}")
                        nc.scalar.activation(
                            out=hb[:], in_=ph[:], func=relu, bias=b1_t[:, m:m + 1]
                        )
                        h_tiles.append(hb)

                    # layer 2: q row = W2^T h  (stationary W2 chunk, M=1)
                    pq = pqp.tile([128, TILE], f32, tag="q")
                    nc.tensor.matmul(
                        out=pq[0:1, :], lhsT=w2_t[:, 0:1], rhs=h_tiles[0][:],
                        start=True, stop=False,
                    )
                    nc.tensor.matmul(
                        out=pq[0:1, :], lhsT=w2_t[:, 1:2], rhs=h_tiles[1][:],
                        start=False, stop=True,
                    )

                    if t % QCHUNK == 0:
                        q_sb = qp.tile([1, QCHUNK * TILE], f32, tag="qs")
                    off = (t % QCHUNK) * TILE
                    nc.vector.tensor_scalar_add(
                        out=q_sb[0:1, off:off + TILE], in0=pq[0:1, :],
                        scalar1=b2_t[0:1, 0:1],
                    )
                    if t % QCHUNK == QCHUNK - 1 or t == TILES - 1:
                        base = (t - t % QCHUNK) * TILE
                        nc.sync.dma_start(
                            out=q_d.ap()[:, base:base + (t % QCHUNK) * TILE + TILE],
                            in_=q_sb[0:1, 0:(t % QCHUNK) * TILE + TILE],
                        )

    nc.compile()
    return nc


def _prep_core(src, dst, attr, tok):
    """Bucket one core's edges; returns idx tiles, attr layout, slot map."""
    tsrc = tok[src]
    tdst = tok[dst]
    gid = (tsrc >= SPLIT).astype(np.int64) * 2 + (tdst >= SPLIT)
    order = np.argsort(gid, kind="stable")
    counts = np.bincount(gid, minlength=4)
    caps = np.array(CAPS) * TILE
    if np.any(counts > caps):
        raise ValueError(f"bucket overflow: {counts} vs {caps}")
    starts = np.concatenate([[0], np.cumsum(caps)[:-1]])
    # slot for each edge (in 'order' sequence)
    within = np.arange(len(src)) - np.concatenate([[0], np.cumsum(counts)[:-1]])[gid[order]]
    slots = starts[gid[order]] + within
    es = np.asarray(order)

    slot_src = np.zeros(SLOTS, np.int64)
    slot_dst = np.zeros(SLOTS, np.int64)
    slot_attr = np.zeros((SLOTS, EDGE_IN), np.float32)
    slot_src[slots] = tsrc[es]
    slot_dst[slots] = tdst[es]
    slot_attr[slots] = attr[es]
    slot_src[slot_src >= SPLIT] -= SPLIT
    slot_dst[slot_dst >= SPLIT] -= SPLIT

    def idx_tile(vals):
        # slot i of tile t -> [i%16, t*32 + i//16], replicated over 8 groups
        v = vals.reshape(TILES, TILE // 16, 16).transpose(2, 0, 1).reshape(16, -1)
        return np.tile(v.astype(np.int16), (8, 1))

    return idx_tile(slot_src), idx_tile(slot_dst), np.ascontiguousarray(slot_attr.T), slots, es


def kernel(**inputs):
    from concourse import bass_utils

    node_emb = np.asarray(inputs["node_emb"], np.float32)
    global_ctx = np.asarray(inputs["global_ctx"], np.float32)
    edge_attr = np.asarray(inputs["edge_attr"], np.float32)
    W1 = np.asarray(inputs["W1"], np.float32)
    b1 = np.asarray(inputs["b1"], np.float32)
    W2 = np.asarray(inputs["W2"], np.float32)
    b2 = np.asarray(inputs["b2"], np.float32)
    edge_index = np.asarray(inputs["edge_index"], np.int64)
    batch = np.asarray(inputs["batch"], np.int64)

    if "nc" not in _CACHE:
        _CACHE["nc"] = _build_nc()
    nc = _CACHE["nc"]

    n = np.arange(VP, dtype=np.int64)
    tok = (n % KR) * 128 + n // KR          # node id -> token id

    node_pad = np.zeros((VP, EMB), np.float32)
    node_pad[:V] = node_emb
    node_flat = np.ascontiguousarray(node_pad.reshape(128, KR * EMB))
    batch_pad = np.zeros(VP, np.int32)
    batch_pad[:V] = batch.astype(np.int32)
    batch_flat = np.ascontiguousarray(batch_pad.reshape(128, KR))

    in_maps = []
    slot_infos = []
    for c in range(NCORES):
        lo, hi = c * E_CORE, (c + 1) * E_CORE
        isrc, idst, attr_t, slots, es = _prep_core(
            edge_index[0, lo:hi], edge_index[1, lo:hi], edge_attr[lo:hi], tok
        )
        slot_infos.append((slots, es))
        in_maps.append({
            "node": node_flat,
            "batchb": batch_flat,
            "gctx": global_ctx,
            "w1": W1,
            "b1": b1,
            "w2": np.ascontiguousarray(W2.reshape(HIDDEN)),
            "b2": b2,
            "isrc": isrc,
            "idst": idst,
            "attr": attr_t,
        })

    _CACHE["last_in_maps"] = in_maps
    res = bass_utils.run_bass_kernel_spmd(nc, in_maps, core_ids=list(range(NCORES)))

    q_full = np.zeros(E, np.float32)
    for c in range(NCORES):
        slots, es = slot_infos[c]
        qrow = np.asarray(res.results[c]["q"]).reshape(SLOTS)
        q_full[c * E_CORE + es] = qrow[slots]
    return q_full
